# revision 1
# baseline (speedup 1.0000x reference)
"""HypergraphConv (node->edge->node message passing) on 8 Trainium2 NeuronCores.

Self-contained Trainium kernel for:
    xw   = x @ W
    m_e  = (1/deg_e) * sum_{k: edge[k]=e} xw[src[k]]
    o_i  = (1/deg_i) * sum_{k: src[k]=i} m_{edge[k]} + bias
    out  = mean_i relu(o_i)                       # [128]

Sharding: nodes are split across the 8 cores (6250 each). Each core owns the
incidence entries whose src node falls in its shard; those entries drive both
the node->edge scatter (partial m, AllReduced across cores) and the
edge->node scatter (complete rows for the core's nodes).

Scatters are done as one-hot matmuls over sorted-and-padded entry streams;
gathers use the SWDGE dma_gather engine against hi/lo-bf16 row tables
(512-byte rows, ~fp32 precision via a hi+lo split accumulated in PSUM).
"""

import numpy as np
import ml_dtypes
from contextlib import ExitStack

import concourse.bacc as bacc
import concourse.bass as bass
import concourse.mybir as mybir
import concourse.tile as tile
from concourse import library_config
from concourse.bass_utils import run_bass_kernel_spmd

NCORES = 8
P = 128

# Problem sizes (overridable for small-scale testing via _configure()).
N_NODES = 50000
N_EDGES = 20000
IN_DIM = 256
OUT_DIM = 128

BF16 = mybir.dt.bfloat16
F32 = mybir.dt.float32
I16 = mybir.dt.int16

PAD_OH = 200.0  # one-hot index for padding entries: matches no iota column


def _derived():
    npc = N_NODES // NCORES
    n_node_tiles = (npc + P - 1) // P
    n_edge_tiles = (N_EDGES + P - 1) // P
    return npc, n_node_tiles, n_edge_tiles


def _wrap_idx16(idx):
    """[L] int -> [128, L//16] int16 SWDGE index layout (16-wrap, x8 replicas)."""
    a = np.asarray(idx, dtype=np.int16).reshape(-1, 16).T
    return np.ascontiguousarray(np.tile(a, (8, 1)))


def _oh_cols(oh):
    """[L] float -> [128, L//128] bf16: column c holds entries c*128..c*128+127."""
    return np.ascontiguousarray(oh.reshape(-1, P).T.astype(ml_dtypes.bfloat16))


def _bucket_entries(gidx, tid, n_tiles, chunks, pad_row):
    """Lay out (gather idx, one-hot idx) entry streams grouped by tile.

    gidx: per-entry gather row index; tid: per-entry tile id;
    chunks[t]: number of 128-entry chunks allotted to tile t (static,
    shared across cores). Returns (gather_idx[L], onehot[L]) padded streams.
    """
    order = np.argsort(tid, kind="stable")
    gidx = gidx[order]
    tid_s = tid[order]
    counts = np.bincount(tid_s, minlength=n_tiles)
    starts = np.concatenate([[0], np.cumsum(counts[:-1])])
    dest_base = np.concatenate([[0], np.cumsum(chunks[:-1])]) * P
    L = int(chunks.sum()) * P
    g_out = np.full(L, pad_row, dtype=np.int64)
    oh_out = np.full(L, PAD_OH, dtype=np.float32)
    n = gidx.shape[0]
    rank = np.arange(n, dtype=np.int64) - starts[tid_s]
    dest = dest_base[tid_s] + rank
    g_out[dest] = gidx
    # one-hot index = original row id within its tile; recomputed by caller
    return g_out, oh_out, dest, order, L


def build_kernel(chunks1, chunks2, last_nt):
    """Build the SPMD device program.

    chunks1[t]: #chunks for edge tile t (phase 1); chunks2[tt]: #chunks for
    node tile tt (phase 2); last_nt: node count of the last node tile.
    """
    npc, n_node_tiles, n_edge_tiles = _derived()
    et_rows = n_edge_tiles * P
    LA = int(np.sum(chunks1)) * P
    LB = int(np.sum(chunks2)) * P
    NCA = LA // P  # total phase-1 chunks
    NCB = LB // P

    nc = bacc.Bacc("TRN2", num_devices=NCORES)

    xT_in = nc.dram_tensor("xT", [IN_DIM, npc], F32, kind="ExternalInput")
    w_in = nc.dram_tensor("w", [IN_DIM, OUT_DIM], F32, kind="ExternalInput")
    bias_in = nc.dram_tensor("bias", [1, OUT_DIM], F32, kind="ExternalInput")
    idxA_in = nc.dram_tensor("idxA", [P, LA // 16], I16, kind="ExternalInput")
    ohA_in = nc.dram_tensor("ohA", [P, NCA], BF16, kind="ExternalInput")
    idxB_in = nc.dram_tensor("idxB", [P, LB // 16], I16, kind="ExternalInput")
    ohB_in = nc.dram_tensor("ohB", [P, NCB], BF16, kind="ExternalInput")
    out_part = nc.dram_tensor("out_part", [OUT_DIM, 1], F32, kind="ExternalOutput")

    import os
    dbg_xwhl = os.environ.get("DBG_XWHL_INPUT") == "1"
    dbg_stop = os.environ.get("DBG_STOP", "")
    if dbg_xwhl:
        xwhl = nc.dram_tensor("xwhl_in", [npc + P, 2 * OUT_DIM], BF16, kind="ExternalInput")
    else:
        xwhl = nc.dram_tensor("xwhl", [npc + P, 2 * OUT_DIM], BF16)  # zero row at npc
    m_part = nc.dram_tensor("m_part", [et_rows, OUT_DIM], F32)
    m_red = nc.dram_tensor("m_red", [et_rows, OUT_DIM], F32, addr_space="Shared")
    deg_part = nc.dram_tensor("deg_part", [P, n_edge_tiles], F32)
    deg_red = nc.dram_tensor("deg_red", [P, n_edge_tiles], F32, addr_space="Shared")
    mtab = nc.dram_tensor("mtab", [et_rows + P, 2 * OUT_DIM], BF16)

    with tile.TileContext(nc) as tc, ExitStack() as ctx:
        pin = ctx.enter_context(tc.tile_pool(name="pin", bufs=1))

        nc.gpsimd.load_library(library_config.mlp)

        # ---- persistent small tiles -------------------------------------
        iota_i = pin.tile([P, P], I16)
        iota_bf = pin.tile([P, P], BF16)
        nc.gpsimd.iota(iota_i[:], [[1, P]], channel_multiplier=0)
        nc.vector.tensor_copy(out=iota_bf[:], in_=iota_i[:])
        ones_bf = pin.tile([P, 1], BF16)
        nc.vector.memset(ones_bf[:], 1.0)
        ones_f32 = pin.tile([P, 1], F32)
        nc.vector.memset(ones_f32[:], 1.0)
        bias_bc = pin.tile([P, OUT_DIM], F32)
        nc.sync.dma_start(out=bias_bc[:], in_=bass.AP(bias_in, 0, [[0, P], [1, OUT_DIM]]))
        acc = pin.tile([P, OUT_DIM], F32)
        nc.vector.memset(acc[:], 0.0)
        deg_sb = pin.tile([P, n_edge_tiles], F32)
        # index/one-hot streams (loaded up front, used by phases 1/2)
        idxA = pin.tile([P, LA // 16], I16)
        ohA = pin.tile([P, NCA], BF16)
        idxB = pin.tile([P, LB // 16], I16)
        ohB = pin.tile([P, NCB], BF16)
        nc.sync.dma_start(out=idxA[:], in_=idxA_in[:])
        nc.sync.dma_start(out=ohA[:], in_=ohA_in[:])
        nc.sync.dma_start(out=idxB[:], in_=idxB_in[:])
        nc.sync.dma_start(out=ohB[:], in_=ohB_in[:])

        def s_build(S_tile, oh_tile, col0, k):
            """S[p, c*128+j] = (oh[p, col0+c] == j), one DVE op for k chunks."""
            s_ap = S_tile[:].rearrange("p (k j) -> p k j", k=k)
            o = oh_tile[:, col0:col0 + k]
            in0 = bass.AP(o.tensor, o.offset, [list(o.ap[0]), list(o.ap[1]), [0, P]])
            it = iota_bf[:]
            in1 = bass.AP(it.tensor, it.offset, [list(it.ap[0]), [0, k], [1, P]])
            nc.vector.tensor_tensor(out=s_ap, in0=in0, in1=in1, op=mybir.AluOpType.is_equal)

        def hilo(dst_bf, src_psum, tmp_pool, nt=P):
            """dst[:, :F] = bf16(src); dst[:, F:] = bf16(src - fp32(hi))."""
            nc.vector.tensor_copy(out=dst_bf[:nt, :OUT_DIM], in_=src_psum[:nt])
            h32 = tmp_pool.tile([P, OUT_DIM], F32, tag="h32", name="h32")
            nc.vector.tensor_copy(out=h32[:nt], in_=dst_bf[:nt, :OUT_DIM])
            nc.vector.tensor_tensor(
                out=dst_bf[:nt, OUT_DIM:], in0=src_psum[:nt], in1=h32[:nt],
                op=mybir.AluOpType.subtract)

        # ---- stage A: xw = x @ W, hi/lo table ---------------------------
        if dbg_xwhl:
            pass
        else:
          with tc.tile_pool(name="pa", bufs=1) as pa, \
             tc.tile_pool(name="pa2", bufs=3) as pa2, \
             tc.tile_pool(name="psa", bufs=2, space="PSUM") as psa:
            kh = IN_DIM // P  # k-halves
            xT_sb = [pa.tile([P, npc], F32, tag=f"xT{k}", name=f"xT{k}") for k in range(kh)]
            w_sb = [pa.tile([P, OUT_DIM], F32, tag=f"w{k}", name=f"wsb{k}") for k in range(kh)]
            for k in range(kh):
                nc.sync.dma_start(out=xT_sb[k][:], in_=xT_in[k * P:(k + 1) * P, :])
                nc.sync.dma_start(out=w_sb[k][:], in_=w_in[k * P:(k + 1) * P, :])
            zrow = pa.tile([P, 2 * OUT_DIM], BF16)
            nc.vector.memset(zrow[:], 0.0)
            nc.sync.dma_start(out=xwhl[npc:npc + P, :], in_=zrow[:])
            for i in range(0, npc, P):
                nt = min(P, npc - i)
                pxw = psa.tile([P, OUT_DIM], F32, tag="pxw")
                for k in range(kh):
                    nc.tensor.matmul(
                        out=pxw[:nt], lhsT=xT_sb[k][:, i:i + nt], rhs=w_sb[k][:],
                        start=(k == 0), stop=(k == kh - 1))
                st = pa2.tile([P, 2 * OUT_DIM], BF16, tag="xst")
                hilo(st, pxw, pa2, nt)
                nc.sync.dma_start(out=xwhl[i:i + nt, :], in_=st[:nt, :])

        # ---- stage B: phase-1 scatter (node -> edge) --------------------
        with tc.tile_pool(name="pb", bufs=3) as pb, \
             tc.tile_pool(name="psb", bufs=2, space="PSUM") as psb, \
             tc.tile_pool(name="psbd", bufs=2, space="PSUM") as psbd:
            cbase = 0
            for t in range(n_edge_tiles):
                kt = int(chunks1[t])
                ni = kt * P
                G = pb.tile([P, kt, 2 * OUT_DIM], BF16, tag="G")
                for g0 in range(0, kt, 64):
                    gk = min(64, kt - g0)
                    nc.gpsimd.dma_gather(
                        G[:, g0:g0 + gk, :], xwhl[:, :],
                        idxA[:, (cbase + g0) * 8:(cbase + g0 + gk) * 8],
                        gk * P, gk * P, 2 * OUT_DIM, single_packet=False)
                S = pb.tile([P, kt * P], BF16, tag="S")
                s_build(S, ohA, cbase, kt)
                pm = psb.tile([P, 2 * OUT_DIM], F32, tag="pm")
                pdeg = psbd.tile([P, 1], F32, tag="pdeg")
                for c in range(kt):
                    nc.tensor.matmul(
                        out=pm[:], lhsT=S[:, c * P:(c + 1) * P], rhs=G[:, c, :],
                        start=(c == 0), stop=(c == kt - 1), skip_group_check=True)
                    nc.tensor.matmul(
                        out=pdeg[:], lhsT=S[:, c * P:(c + 1) * P], rhs=ones_bf[:],
                        start=(c == 0), stop=(c == kt - 1), skip_group_check=True)
                mt = pb.tile([P, OUT_DIM], F32, tag="mt")
                nc.vector.tensor_copy(out=mt[:], in_=pm[:, :OUT_DIM])
                nc.vector.tensor_tensor(
                    out=mt[:], in0=mt[:], in1=pm[:, OUT_DIM:], op=mybir.AluOpType.add)
                nc.vector.tensor_copy(out=deg_sb[:, t:t + 1], in_=pdeg[:])
                nc.sync.dma_start(out=m_part[t * P:(t + 1) * P, :], in_=mt[:])
                cbase += kt
            nc.sync.dma_start(out=deg_part[:], in_=deg_sb[:])

        if dbg_stop == "B":
            with tc.tile_pool(name="pz", bufs=1) as pz:
                oc = pz.tile([P, 1], F32)
                nc.vector.tensor_copy(out=oc[:], in_=deg_sb[:, 0:1])
                nc.sync.dma_start(out=out_part[:, :], in_=oc[:])

        # ---- stage C: AllReduce + m' table ------------------------------
        dbg_no_cc = os.environ.get("DBG_NO_CC") == "1"
        if dbg_stop != "B":
          if dbg_no_cc:
            nc.sync.dma_start(out=m_red[:, :], in_=m_part[:, :])
            nc.sync.dma_start(out=deg_red[:, :], in_=deg_part[:, :])
          else:
            nc.gpsimd.collective_compute(
              "AllReduce", mybir.AluOpType.add, replica_groups=[list(range(NCORES))],
              ins=[m_part[:, :]], outs=[m_red[:, :]])
            nc.gpsimd.collective_compute(
              "AllReduce", mybir.AluOpType.add, replica_groups=[list(range(NCORES))],
              ins=[deg_part[:, :]], outs=[deg_red[:, :]])

          with tc.tile_pool(name="pc", bufs=3) as pc:
              dga = pc.tile([P, n_edge_tiles], F32, tag="dga")
              nc.sync.dma_start(out=dga[:], in_=deg_red[:])
              binv = pin.tile([P, n_edge_tiles], F32)
              nc.vector.tensor_scalar(
                  out=binv[:], in0=dga[:], scalar1=1.0, scalar2=None,
                  op0=mybir.AluOpType.max)
              nc.vector.reciprocal(out=binv[:], in_=binv[:])
              zrow2 = pc.tile([P, 2 * OUT_DIM], BF16, tag="zr2")
              nc.vector.memset(zrow2[:], 0.0)
              nc.sync.dma_start(out=mtab[et_rows:et_rows + P, :], in_=zrow2[:])
              for t in range(n_edge_tiles):
                  mt = pc.tile([P, OUT_DIM], F32, tag="mtc")
                  nc.sync.dma_start(out=mt[:], in_=m_red[t * P:(t + 1) * P, :])
                  nc.vector.tensor_scalar(
                      out=mt[:], in0=mt[:], scalar1=binv[:, t:t + 1], scalar2=None,
                      op0=mybir.AluOpType.mult)
                  st = pc.tile([P, 2 * OUT_DIM], BF16, tag="mst")
                  hilo(st, mt, pc)
                  nc.sync.dma_start(out=mtab[t * P:(t + 1) * P, :], in_=st[:])

        if dbg_stop == "C":
            with tc.tile_pool(name="pz2", bufs=1) as pz2:
                oc2 = pz2.tile([P, 1], F32)
                nc.vector.tensor_copy(out=oc2[:], in_=binv[:, 0:1])
                nc.sync.dma_start(out=out_part[:, :], in_=oc2[:])

        # ---- stage D: phase-2 scatter (edge -> node) + post -------------
        run_d = dbg_stop not in ("B", "C")
        run_e = run_d and dbg_stop != "D"
        d_lvl = int(os.environ.get("DBG_D_LVL", "4"))

        if run_d:
          with tc.tile_pool(name="pd", bufs=3) as pd, \
             tc.tile_pool(name="psd", bufs=2, space="PSUM") as psd, \
             tc.tile_pool(name="psdd", bufs=2, space="PSUM") as psdd:
            cbase = 0
            for tt in range(n_node_tiles):
                kt = int(chunks2[tt])
                ni = kt * P
                nt = last_nt if tt == n_node_tiles - 1 else P
                G = pd.tile([P, kt, 2 * OUT_DIM], BF16, tag="G2")
                for g0 in range(0, kt, 64):
                    gk = min(64, kt - g0)
                    nc.gpsimd.dma_gather(
                        G[:, g0:g0 + gk, :], mtab[:, :],
                        idxB[:, (cbase + g0) * 8:(cbase + g0 + gk) * 8],
                        gk * P, gk * P, 2 * OUT_DIM, single_packet=False)
                if d_lvl <= 1:
                    nc.vector.tensor_copy(out=acc[:, tt:tt + 1], in_=G[:, 0, 0:1])
                    cbase += kt
                    continue
                S = pd.tile([P, kt * P], BF16, tag="S2")
                s_build(S, ohB, cbase, kt)
                if d_lvl <= 2:
                    nc.vector.tensor_copy(out=acc[:, tt:tt + 1], in_=S[:, 0:1])
                    cbase += kt
                    continue
                po = psd.tile([P, 2 * OUT_DIM], F32, tag="po")
                pdeg = psdd.tile([P, 1], F32, tag="pdeg2")
                for c in range(kt):
                    nc.tensor.matmul(
                        out=po[:], lhsT=S[:, c * P:(c + 1) * P], rhs=G[:, c, :],
                        start=(c == 0), stop=(c == kt - 1), skip_group_check=True)
                    nc.tensor.matmul(
                        out=pdeg[:], lhsT=S[:, c * P:(c + 1) * P], rhs=ones_bf[:],
                        start=(c == 0), stop=(c == kt - 1), skip_group_check=True)
                if d_lvl <= 3:
                    nc.vector.tensor_copy(out=acc[:, tt:tt + 1], in_=po[:, 0:1])
                    cbase += kt
                    continue
                dinv = pd.tile([P, 1], F32, tag="dinv")
                nc.vector.tensor_scalar(
                    out=dinv[:], in0=pdeg[:], scalar1=1.0, scalar2=None,
                    op0=mybir.AluOpType.max)
                nc.vector.reciprocal(out=dinv[:], in_=dinv[:])
                ot = pd.tile([P, OUT_DIM], F32, tag="ot")
                nc.vector.tensor_copy(out=ot[:nt], in_=po[:nt, :OUT_DIM])
                nc.vector.tensor_tensor(
                    out=ot[:nt], in0=ot[:nt], in1=po[:nt, OUT_DIM:],
                    op=mybir.AluOpType.add)
                nc.vector.tensor_scalar(
                    out=ot[:nt], in0=ot[:nt], scalar1=dinv[:nt, :1], scalar2=None,
                    op0=mybir.AluOpType.mult)
                nc.vector.tensor_tensor(
                    out=ot[:nt], in0=ot[:nt], in1=bias_bc[:nt], op=mybir.AluOpType.add)
                nc.vector.tensor_scalar(
                    out=ot[:nt], in0=ot[:nt], scalar1=0.0, scalar2=None,
                    op0=mybir.AluOpType.max)
                nc.vector.tensor_tensor(
                    out=acc[:nt], in0=acc[:nt], in1=ot[:nt], op=mybir.AluOpType.add)
                cbase += kt
            if dbg_stop == "D":
                oc3 = pd.tile([P, 1], F32, tag="oc3", name="oc3")
                nc.vector.tensor_copy(out=oc3[:], in_=acc[:, 0:1])
                nc.sync.dma_start(out=out_part[:, :], in_=oc3[:])

        # ---- stage E: column sum over nodes -> [OUT_DIM, 1] -------------
        if run_e:
          with tc.tile_pool(name="pe", bufs=1) as pe, \
             tc.tile_pool(name="pse", bufs=1, space="PSUM") as pse:
            pcol = pse.tile([P, 1], F32)
            nc.tensor.matmul(out=pcol[:OUT_DIM], lhsT=acc[:], rhs=ones_f32[:],
                             start=True, stop=True)
            ocol = pe.tile([P, 1], F32)
            nc.vector.tensor_copy(out=ocol[:OUT_DIM], in_=pcol[:OUT_DIM])
            nc.sync.dma_start(out=out_part[:, :], in_=ocol[:OUT_DIM])

    nc.compile()
    return nc


def prepare_inputs(x, w, bias, hyperedge_index):
    """Host-side sharding: split entries by src-node shard, sort/pad both
    phase streams, compute the static chunk structure shared by all cores."""
    npc, n_node_tiles, n_edge_tiles = _derived()
    src = np.asarray(hyperedge_index[0], dtype=np.int64)
    edge = np.asarray(hyperedge_index[1], dtype=np.int64)

    core_of = src // npc
    per_core = []
    for c in range(NCORES):
        sel = core_of == c
        per_core.append((src[sel] - c * npc, edge[sel]))

    # static chunk structure = max over cores, per tile
    cnt1 = np.zeros((NCORES, n_edge_tiles), np.int64)
    cnt2 = np.zeros((NCORES, n_node_tiles), np.int64)
    for c, (s_loc, e_glob) in enumerate(per_core):
        cnt1[c] = np.bincount(e_glob // P, minlength=n_edge_tiles)
        cnt2[c] = np.bincount(s_loc // P, minlength=n_node_tiles)
    chunks1 = np.maximum(1, -(-cnt1.max(axis=0) // P))
    chunks2 = np.maximum(1, -(-cnt2.max(axis=0) // P))

    in_maps = []
    for c, (s_loc, e_glob) in enumerate(per_core):
        # phase 1: group by edge tile; gather xwhl[s_loc], one-hot = edge%P
        t1 = e_glob // P
        g1, oh1, dest1, order1, LA = _bucket_entries(s_loc, t1, n_edge_tiles, chunks1, npc)
        oh1[dest1] = (e_glob % P)[order1].astype(np.float32)
        # phase 2: group by node tile; gather mtab[e_glob], one-hot = s_loc%P
        t2 = s_loc // P
        g2, oh2, dest2, order2, LB = _bucket_entries(
            e_glob, t2, n_node_tiles, chunks2, n_edge_tiles * P)
        oh2[dest2] = (s_loc % P)[order2].astype(np.float32)

        xT = np.ascontiguousarray(x[c * npc:(c + 1) * npc].T.astype(np.float32))
        in_maps.append({
            "xT": xT,
            "w": np.ascontiguousarray(w.astype(np.float32)),
            "bias": np.ascontiguousarray(bias.astype(np.float32)).reshape(1, -1),
            "idxA": _wrap_idx16(g1),
            "ohA": _oh_cols(oh1),
            "idxB": _wrap_idx16(g2),
            "ohB": _oh_cols(oh2),
        })

    last_nt = npc - (n_node_tiles - 1) * P
    return in_maps, chunks1, chunks2, last_nt


def kernel(x_node_features, lin_weight, bias, hyperedge_index):
    in_maps, chunks1, chunks2, last_nt = prepare_inputs(
        x_node_features, lin_weight, bias, hyperedge_index)
    nc = build_kernel(chunks1, chunks2, last_nt)
    res = run_bass_kernel_spmd(nc, in_maps, list(range(NCORES)))
    total = np.zeros(OUT_DIM, np.float64)
    for c in range(NCORES):
        total += res.results[c]["out_part"][:, 0].astype(np.float64)
    return (total / N_NODES).astype(np.float32)



# revision 9
# speedup vs baseline: 1.3750x; 1.3750x over previous
"""HypergraphConv v5: v3 + half-split phase-1 collectives.

Phase B runs in two edge-space halves; each half's ReduceScatter/rescale/
AllGather issues as soon as its partials are written, so the first half's
collective latency overlaps the second half's gather/compute work.

All stage pools open once per rep (shared G pool for both scatter phases,
fixed-size G tiles), removing the per-stage pool drain barriers that
serialized the v2 timeline.
"""

import numpy as np
import ml_dtypes
from contextlib import ExitStack

import concourse.bacc as bacc
import concourse.bass as bass
import concourse.mybir as mybir
import concourse.tile as tile
from concourse import library_config
from concourse.bass_utils import run_bass_kernel_spmd

NCORES = 8
P = 128

N_NODES = 50000
N_EDGES = 20000
IN_DIM = 256
OUT_DIM = 128

W1 = 64
W2 = 64
GROUP = 16

BF16 = mybir.dt.bfloat16
F32 = mybir.dt.float32
I16 = mybir.dt.int16

PAD_OH = 200.0


def _derived():
    npc = N_NODES // NCORES
    e_pad = -(-N_EDGES // (NCORES * P)) * (NCORES * P)
    n_t1 = e_pad // W1
    eslice = e_pad // NCORES
    n_t2 = -(-npc // W2)
    return npc, e_pad, n_t1, eslice, n_t2


def _wrap_idx16(idx):
    a = np.asarray(idx, dtype=np.int16).reshape(-1, 16).T
    return np.ascontiguousarray(np.tile(a, (8, 1)))


def _oh_cols(oh):
    return np.ascontiguousarray(oh.reshape(-1, P).T.astype(ml_dtypes.bfloat16))


def _bucket_entries(gidx, oh, tid, n_tiles, chunks):
    order = np.argsort(tid, kind="stable")
    gidx = gidx[order]
    oh = oh[order]
    tid_s = tid[order]
    counts = np.bincount(tid_s, minlength=n_tiles)
    starts = np.concatenate([[0], np.cumsum(counts[:-1])])
    dest_base = np.concatenate([[0], np.cumsum(chunks[:-1])]) * P
    L = int(chunks.sum()) * P
    g_out = np.zeros(L, dtype=np.int64)
    oh_out = np.full(L, PAD_OH, dtype=np.float32)
    n = gidx.shape[0]
    rank = np.arange(n, dtype=np.int64) - starts[tid_s]
    dest = dest_base[tid_s] + rank
    g_out[dest] = gidx
    oh_out[dest] = oh
    return g_out, oh_out, L


def _make_groups(chunks):
    groups = []
    t = 0
    n_tiles = len(chunks)
    cbase = 0
    while t < n_tiles:
        nch = 0
        t0 = t
        while t < n_tiles and (nch == 0 or nch + chunks[t] <= GROUP):
            nch += int(chunks[t])
            t += 1
        groups.append((t0, t, cbase, nch))
        cbase += nch
    return groups


def build_kernel(chunks1, chunks2, bias_nz, reps=1):
    import os
    n_queues = int(os.environ.get("V2_QUEUES", "4"))
    gbufs = int(os.environ.get("V2_GBUFS", "20"))
    global GROUP
    GROUP = int(os.environ.get("V2_GROUP", "16"))
    npc, e_pad, n_t1, eslice, n_t2 = _derived()
    LA = int(np.sum(chunks1)) * P
    LB = int(np.sum(chunks2)) * P
    NCA = LA // P
    NCB = LB // P
    n_t1_half = n_t1 // 2
    groups1_h = []
    cb = 0
    for h in range(2):
        ch = chunks1[h * n_t1_half:(h + 1) * n_t1_half]
        gs = [(t_lo + h * n_t1_half, t_hi + h * n_t1_half, gc0 + cb, nch)
              for (t_lo, t_hi, gc0, nch) in _make_groups(ch)]
        groups1_h.append(gs)
        cb += int(np.sum(ch))
    groups2 = _make_groups(chunks2)

    nc = bacc.Bacc("TRN2", num_devices=NCORES, num_swdge_queues=n_queues)

    xT_in = nc.dram_tensor("xT", [IN_DIM, npc], BF16, kind="ExternalInput")
    w_in = nc.dram_tensor("w", [IN_DIM, OUT_DIM], BF16, kind="ExternalInput")
    bias_in = nc.dram_tensor("bias", [1, OUT_DIM], F32, kind="ExternalInput")
    idxA_in = nc.dram_tensor("idxA", [P, LA // 16], I16, kind="ExternalInput")
    ohA_in = nc.dram_tensor("ohA", [P, NCA], BF16, kind="ExternalInput")
    idxB_in = nc.dram_tensor("idxB", [P, LB // 16], I16, kind="ExternalInput")
    ohB_in = nc.dram_tensor("ohB", [P, NCB], BF16, kind="ExternalInput")
    binv_in = nc.dram_tensor("binv", [P, eslice // P], F32, kind="ExternalInput")
    dinv_in = nc.dram_tensor("dinv", [P, n_t2], F32, kind="ExternalInput")
    out_part = nc.dram_tensor("out_part", [OUT_DIM, 1], F32, kind="ExternalOutput")

    xw_tab = nc.dram_tensor("xw_tab", [npc, OUT_DIM], BF16)
    m_part = nc.dram_tensor("m_part", [e_pad, OUT_DIM], F32)
    half_rows = e_pad // 2
    esl_h = half_rows // NCORES
    n_t1_h = n_t1 // 2
    m_red_h = [nc.dram_tensor(f"m_red{h}", [esl_h, OUT_DIM], F32)
               for h in range(2)]
    mtab_s_h = [nc.dram_tensor(f"mtab_s{h}", [esl_h, OUT_DIM], BF16)
                for h in range(2)]
    mtab = nc.dram_tensor("mtab", [e_pad, OUT_DIM], BF16, addr_space="Shared")

    with tile.TileContext(nc) as tc, ExitStack() as ctx:
        pin = ctx.enter_context(tc.tile_pool(name="pin", bufs=1))

        nc.gpsimd.load_library(library_config.mlp)

        iota_i = pin.tile([P, P], I16)
        iota_bf = pin.tile([P, P], BF16)
        nc.gpsimd.iota(iota_i[:], [[1, P]], channel_multiplier=0)
        nc.vector.tensor_copy(out=iota_bf[:], in_=iota_i[:])
        ones_f32 = pin.tile([P, 1], F32)
        nc.vector.memset(ones_f32[:], 1.0)
        binv_sb = pin.tile([P, eslice // P], F32)
        dinv_sb = pin.tile([P, n_t2], F32)
        nc.sync.dma_start(out=binv_sb[:], in_=binv_in[:])
        nc.sync.dma_start(out=dinv_sb[:], in_=dinv_in[:])
        idxA = pin.tile([P, LA // 16], I16)
        ohA = pin.tile([P, NCA], BF16)
        idxB = pin.tile([P, LB // 16], I16)
        ohB = pin.tile([P, NCB], BF16)
        nc.sync.dma_start(out=idxA[:], in_=idxA_in[:])
        nc.sync.dma_start(out=ohA[:], in_=ohA_in[:])
        nc.sync.dma_start(out=idxB[:], in_=idxB_in[:])
        nc.sync.dma_start(out=ohB[:], in_=ohB_in[:])
        if bias_nz:
            bias_bc = pin.tile([P, OUT_DIM], F32)
            nc.sync.dma_start(
                out=bias_bc[:], in_=bass.AP(bias_in, 0, [[0, P], [1, OUT_DIM]]))

        def s_build(S_tile, oh_tile, col0, k, w):
            s_ap = S_tile[:, :k * w].rearrange("p (k j) -> p k j", k=k)
            o = oh_tile[:, col0:col0 + k]
            in0 = bass.AP(o.tensor, o.offset, [list(o.ap[0]), list(o.ap[1]), [0, w]])
            it = iota_bf[:, :w]
            in1 = bass.AP(it.tensor, it.offset, [list(it.ap[0]), [0, k], [1, w]])
            nc.vector.tensor_tensor(out=s_ap, in0=in0, in1=in1,
                                    op=mybir.AluOpType.is_equal)

        qrr = [0]
        SMAX = max(max(int(c) for c in chunks1), max(int(c) for c in chunks2))
        GMAX = max(GROUP, SMAX)

        def one_rep():
            with tc.tile_pool(name="pa", bufs=1) as pa, \
                 tc.tile_pool(name="pw", bufs=3) as pw, \
                 tc.tile_pool(name="pg", bufs=gbufs) as pg, \
                 tc.tile_pool(name="psa", bufs=2, space="PSUM") as psa, \
                 tc.tile_pool(name="psb", bufs=2, space="PSUM") as psb, \
                 tc.tile_pool(name="psc", bufs=1, space="PSUM") as psc:

                def gather_group(tab, idx_sb, c0, nch, tag):
                    G = pg.tile([P, GMAX, OUT_DIM], BF16, tag=tag, name=tag)
                    for g0 in range(0, nch, GROUP):
                        gk = min(GROUP, nch - g0)
                        nc.gpsimd.dma_gather(
                            G[:, g0:g0 + gk, :], tab[:, :],
                            idx_sb[:, (c0 + g0) * 8:(c0 + g0 + gk) * 8],
                            gk * P, gk * P, OUT_DIM, single_packet=False,
                            queue_num=qrr[0])
                        qrr[0] = (qrr[0] + 1) % n_queues
                    return G

                # ---- stage A: xw table = x @ W (bf16) ------------------
                kh = IN_DIM // P
                xT_sb = [pa.tile([P, npc], BF16, tag=f"xT{k}", name=f"xT{k}")
                         for k in range(kh)]
                w_sb = [pa.tile([P, OUT_DIM], BF16, tag=f"w{k}", name=f"w{k}")
                        for k in range(kh)]
                for k in range(kh):
                    nc.sync.dma_start(out=xT_sb[k][:], in_=xT_in[k * P:(k + 1) * P, :])
                    nc.sync.dma_start(out=w_sb[k][:], in_=w_in[k * P:(k + 1) * P, :])
                for i in range(0, npc, P):
                    nt = min(P, npc - i)
                    pxw = psa.tile([P, OUT_DIM], F32, tag="pxw", name="pxw")
                    for k in range(kh):
                        nc.tensor.matmul(
                            out=pxw[:nt], lhsT=xT_sb[k][:, i:i + nt], rhs=w_sb[k][:],
                            start=(k == 0), stop=(k == kh - 1))
                    st = pw.tile([P, OUT_DIM], BF16, tag="xst", name="xst")
                    nc.scalar.copy(out=st[:nt], in_=pxw[:nt])
                    nc.sync.dma_start(out=xw_tab[i:i + nt, :], in_=st[:nt, :])

                # ---- stage B: per-half scatter + RS/scale/AG -----------
                for h in range(2):
                    for (t_lo, t_hi, gc0, nch) in groups1_h[h]:
                        G = gather_group(xw_tab, idxA, gc0, nch, "G")
                        cbase = gc0
                        for t in range(t_lo, t_hi):
                            kt = int(chunks1[t])
                            S = pw.tile([P, SMAX * W1], BF16, tag="S", name="S")
                            s_build(S, ohA, cbase, kt, W1)
                            pm = psb.tile([P, OUT_DIM], F32, tag="pm", name="pm")
                            for c in range(kt):
                                nc.tensor.matmul(
                                    out=pm[:W1],
                                    lhsT=S[:, c * W1:(c + 1) * W1],
                                    rhs=G[:, cbase - gc0 + c, :],
                                    start=(c == 0), stop=(c == kt - 1),
                                    skip_group_check=True)
                            mt = pw.tile([P, OUT_DIM], F32, tag="mt", name="mt")
                            nc.scalar.copy(out=mt[:W1], in_=pm[:W1])
                            nc.sync.dma_start(
                                out=m_part[t * W1:(t + 1) * W1, :], in_=mt[:W1, :])
                            cbase += kt
                    nc.gpsimd.collective_compute(
                        "ReduceScatter", mybir.AluOpType.add,
                        replica_groups=[list(range(NCORES))],
                        ins=[m_part[h * half_rows:(h + 1) * half_rows, :]],
                        outs=[m_red_h[h][:, :]])
                    for ts in range(esl_h // P):
                        mc = pw.tile([P, OUT_DIM], F32, tag="mc", name="mc")
                        nc.sync.dma_start(
                            out=mc[:], in_=m_red_h[h][ts * P:(ts + 1) * P, :])
                        ms = pw.tile([P, OUT_DIM], BF16, tag="ms", name="ms")
                        nc.scalar.activation(
                            out=ms[:], in_=mc[:],
                            func=mybir.ActivationFunctionType.Copy,
                            scale=binv_sb[:, h * (esl_h // P) + ts:
                                          h * (esl_h // P) + ts + 1])
                        nc.sync.dma_start(
                            out=mtab_s_h[h][ts * P:(ts + 1) * P, :], in_=ms[:])
                    nc.gpsimd.collective_compute(
                        "AllGather", mybir.AluOpType.bypass,
                        replica_groups=[list(range(NCORES))],
                        ins=[mtab_s_h[h][:, :]],
                        outs=[mtab[h * half_rows:(h + 1) * half_rows, :]])

                # ---- stage D: edge -> node scatter + relu + node-sum ---
                pcol = psc.tile([P, 1], F32, name="pcol")
                first = True
                for (t_lo, t_hi, gc0, nch) in groups2:
                    G = gather_group(mtab, idxB, gc0, nch, "G")
                    cbase = gc0
                    for tt in range(t_lo, t_hi):
                        kt = int(chunks2[tt])
                        nt = min(W2, npc - tt * W2)
                        S = pw.tile([P, SMAX * W2], BF16, tag="S", name="S")
                        s_build(S, ohB, cbase, kt, W2)
                        po = psb.tile([P, OUT_DIM], F32, tag="pm", name="po")
                        for c in range(kt):
                            nc.tensor.matmul(
                                out=po[:W2],
                                lhsT=S[:, c * W2:(c + 1) * W2],
                                rhs=G[:, cbase - gc0 + c, :],
                                start=(c == 0), stop=(c == kt - 1),
                                skip_group_check=True)
                        rt = pw.tile([P, OUT_DIM], F32, tag="rt", name="rt")
                        if bias_nz:
                            nc.scalar.activation(
                                out=rt[:nt], in_=po[:nt],
                                func=mybir.ActivationFunctionType.Copy,
                                scale=dinv_sb[:nt, tt:tt + 1])
                            nc.vector.tensor_tensor(
                                out=rt[:nt], in0=rt[:nt], in1=bias_bc[:nt],
                                op=mybir.AluOpType.add)
                            nc.vector.tensor_scalar(
                                out=rt[:nt], in0=rt[:nt], scalar1=0.0,
                                scalar2=None, op0=mybir.AluOpType.max)
                        else:
                            nc.scalar.activation(
                                out=rt[:nt], in_=po[:nt],
                                func=mybir.ActivationFunctionType.Relu,
                                scale=dinv_sb[:nt, tt:tt + 1])
                        nc.tensor.matmul(
                            out=pcol[:OUT_DIM], lhsT=rt[:nt, :],
                            rhs=ones_f32[:nt, :],
                            start=first, stop=(tt == n_t2 - 1),
                            skip_group_check=True)
                        first = False
                        cbase += kt
                ocol = pw.tile([P, 1], F32, tag="oc", name="oc")
                nc.vector.tensor_copy(out=ocol[:OUT_DIM], in_=pcol[:OUT_DIM])
                nc.sync.dma_start(out=out_part[:, :], in_=ocol[:OUT_DIM])

        for _rep in range(reps):
            one_rep()

    nc.compile()
    return nc


def prepare_inputs(x, w, bias, hyperedge_index):
    npc, e_pad, n_t1, eslice, n_t2 = _derived()
    src = np.asarray(hyperedge_index[0], dtype=np.int64)
    edge = np.asarray(hyperedge_index[1], dtype=np.int64)

    deg_e = np.bincount(edge, minlength=e_pad).astype(np.float64)
    b_inv = np.where(deg_e > 0, 1.0 / np.maximum(deg_e, 1), 0.0).astype(np.float32)
    deg_n = np.bincount(src, minlength=N_NODES).astype(np.float64)
    d_inv = np.where(deg_n > 0, 1.0 / np.maximum(deg_n, 1), 0.0).astype(np.float32)

    core_of = src // npc
    per_core = []
    cnt1 = np.zeros((NCORES, n_t1), np.int64)
    cnt2 = np.zeros((NCORES, n_t2), np.int64)
    for c in range(NCORES):
        sel = core_of == c
        s_loc, e_glob = src[sel] - c * npc, edge[sel]
        per_core.append((s_loc, e_glob))
        cnt1[c] = np.bincount(e_glob // W1, minlength=n_t1)
        cnt2[c] = np.bincount(s_loc // W2, minlength=n_t2)
    chunks1 = np.maximum(1, -(-cnt1.max(axis=0) // P))
    chunks2 = np.maximum(1, -(-cnt2.max(axis=0) // P))

    bias = np.asarray(bias, dtype=np.float32).reshape(1, -1)
    bias_nz = bool(np.any(bias != 0))
    x = np.asarray(x)
    w_bf = np.ascontiguousarray(np.asarray(w, dtype=np.float32).astype(
        ml_dtypes.bfloat16))

    in_maps = []
    for c, (s_loc, e_glob) in enumerate(per_core):
        g1, oh1, LA = _bucket_entries(
            s_loc, (e_glob % W1).astype(np.float32), e_glob // W1, n_t1, chunks1)
        g2, oh2, LB = _bucket_entries(
            e_glob, (s_loc % W2).astype(np.float32), s_loc // W2, n_t2, chunks2)

        xT = np.ascontiguousarray(
            x[c * npc:(c + 1) * npc].astype(np.float32).T.astype(ml_dtypes.bfloat16))
        esl_h = e_pad // 2 // NCORES
        cols = []
        for h in range(2):
            base = h * (e_pad // 2) + c * esl_h
            cols.append(b_inv[base:base + esl_h].reshape(-1, P).T)
        binv_c = np.ascontiguousarray(np.concatenate(cols, axis=1))
        dinv_c = np.zeros((P, n_t2), np.float32)
        dloc = np.zeros(n_t2 * W2, np.float32)
        dloc[:npc] = d_inv[c * npc:(c + 1) * npc]
        dinv_c[:W2, :] = dloc.reshape(n_t2, W2).T
        in_maps.append({
            "xT": xT,
            "w": w_bf,
            "bias": np.ascontiguousarray(bias),
            "idxA": _wrap_idx16(g1),
            "ohA": _oh_cols(oh1),
            "idxB": _wrap_idx16(g2),
            "ohB": _oh_cols(oh2),
            "binv": binv_c,
            "dinv": np.ascontiguousarray(dinv_c),
        })

    return in_maps, chunks1, chunks2, bias_nz


def kernel(x_node_features, lin_weight, bias, hyperedge_index):
    in_maps, chunks1, chunks2, bias_nz = prepare_inputs(
        x_node_features, lin_weight, bias, hyperedge_index)
    nc = build_kernel(chunks1, chunks2, bias_nz)
    res = run_bass_kernel_spmd(nc, in_maps, list(range(NCORES)))
    total = np.zeros(OUT_DIM, np.float64)
    for c in range(NCORES):
        total += res.results[c]["out_part"][:, 0].astype(np.float64)
    return (total / N_NODES).astype(np.float32)
